# revision 8
# baseline (speedup 1.0000x reference)
"""Trainium2 Bass kernel: batched chamfer-style metric (nn_Metric_56985626083917).

Reference computation per batch b (B=8, N=M=4096, D=3):
    sqd[n,m] = |pred_n - gt_m|^2   (clamped >= 0)
    dist1 = sqrt(min_m sqd)  [N] ; dist2 = sqrt(min_n sqd)  [M]
    loss_b = mean(dist1)+mean(dist2) + 3*(mean(top2048(dist1))+mean(top2048(dist2)))
    out = mean_b loss_b

Strategy: data-parallel, one batch per NeuronCore (8 cores).
Per core the device computes zt[n,m] = -sqd[n,m] via K=13 fp16 matmuls with
error-compensated hi/lo splits (fp32-grade accuracy at full fp16 PE rate):
    zt = sum_c 2*p_c*g_c - |p|^2 - |g|^2

Main loop per 128-row pred tile (32 iterations):
    PE   : 8 matmuls -> PSUM [128, 4096] fp32 (zt tile)
    Act  : copy PSUM -> SBUF fp16 stage (double-buffered; DVE reading PSUM
           directly is far slower than Act on this runtime)
    DVE  : tensor_tensor max (stage, run2) -> running column-max (dist2 path)
           tensor_reduce max (stage)       -> per-row max        (dist1 path)
           both on fp16 SBUF operands (2x/4x DVE perf modes)
Device outputs row maxes [128, 32] fp32 and column maxes [128, 4096] fp16;
the host (O(N) work) finishes the partition fold, does relu/sqrt, means, and
exact top-k via np.partition, then averages the 8 losses.

Timing note: _build_nc(reps) realizes in-NEFF repetition as a hardware
For_i loop, so the device genuinely executes the body `reps` times while
the NEFF stays compact.
"""

import os
import sys

import numpy as np

for _p in ("/opt/trn_rl_repo",):
    if os.path.isdir(_p) and _p not in sys.path:
        sys.path.insert(0, _p)

import concourse.bass as bass  # noqa: E402
import concourse.mybir as mybir  # noqa: E402
import concourse.tile as tile  # noqa: E402
from concourse import bacc  # noqa: E402
from concourse.bass_utils import run_bass_kernel_spmd  # noqa: E402

B = 8
N = 4096  # pred points per batch
M = 4096  # gt points per batch
P = 128  # partitions
KSLOTS = 13
NTILE = N // P  # 32
PSHALF = 2048  # gt columns per PSUM half (2 halves overlap PE/Act)
NCHUNK = M // PSHALF  # 2
MM_N = 512  # moving free dim per matmul (<= 1 PSUM bank)
K1 = N // 2  # top-k count (PERCENT=0.5)
WEIGHT = 3.0

F16 = mybir.dt.float16
F32 = mybir.dt.float32
Alu = mybir.AluOpType

LAST_RESULT = None
_CACHE = {}


def _build_nc(reps=1):
    nc = bacc.Bacc(
        "TRN2", target_bir_lowering=False, debug=False, num_devices=B
    )
    a_in = nc.dram_tensor("A", [KSLOTS, N], F16, kind="ExternalInput")
    g_in = nc.dram_tensor("G", [KSLOTS, M], F16, kind="ExternalInput")
    r1_out = nc.dram_tensor("R1OUT", [P, NTILE], F32, kind="ExternalOutput")
    c2_out = nc.dram_tensor("C2OUT", [P, M], F16, kind="ExternalOutput")

    with tile.TileContext(nc) as tc:
        with tc.For_i(0, reps, 1):
            _body(nc, tc, a_in, g_in, r1_out, c2_out)
    nc.compile()
    return nc


def _body(nc, tc, a_in, g_in, r1_out, c2_out):
    from contextlib import ExitStack

    with ExitStack() as ctx:
        runp = ctx.enter_context(tc.tile_pool(name="run", bufs=1))

        A = runp.tile([KSLOTS, N], F16)
        G = runp.tile([KSLOTS, M], F16)
        nc.sync.dma_start(out=A, in_=a_in[:])
        nc.sync.dma_start(out=G, in_=g_in[:])

        run2 = runp.tile([P, M], F16)
        Rpart = runp.tile([P, NTILE], F32, name="Rpart")

        # ---------- main loop ----------
        # PSUM halves (bufs=2) let PE fill half jj+1 while Act drains half jj;
        # Act assembles both halves into one full-tile fp16 stage so DVE keeps
        # one tensor_tensor + one tensor_reduce per 128-row tile.
        with tc.tile_pool(name="ps_main", bufs=2, space="PSUM") as psum, \
             tc.tile_pool(name="stage", bufs=3) as stgp:
            for i in range(NTILE):
                stg = stgp.tile([P, M], F16)
                for jj in range(NCHUNK):
                    ps = psum.tile([P, PSHALF], F32)
                    for kk in range(PSHALF // MM_N):
                        nc.tensor.matmul(
                            ps[:, kk * MM_N : (kk + 1) * MM_N],
                            A[:, i * P : (i + 1) * P],
                            G[:, jj * PSHALF + kk * MM_N : jj * PSHALF + (kk + 1) * MM_N],
                            start=True,
                            stop=True,
                        )
                    nc.scalar.copy(
                        out=stg[:, jj * PSHALF : (jj + 1) * PSHALF], in_=ps
                    )
                # colmax accumulate (dist2 path); tile 0 initializes run2
                # via max(stg, stg) = stg, so no memset is needed
                if i == 0:
                    nc.vector.tensor_tensor(run2, stg, stg, op=Alu.max)
                else:
                    nc.vector.tensor_tensor(run2, stg, run2, op=Alu.max)
                # rowmax (dist1 path)
                nc.vector.tensor_reduce(
                    out=Rpart[:, i : i + 1],
                    in_=stg,
                    axis=mybir.AxisListType.X,
                    op=Alu.max,
                )

        nc.sync.dma_start(out=r1_out[:], in_=Rpart)
        nc.sync.dma_start(out=c2_out[:], in_=run2)


def _split16(x):
    hi = x.astype(np.float16)
    lo = (x - hi.astype(np.float64)).astype(np.float16)
    return hi, lo


def _prep(pred, gt):
    """Build the [13, 4096] fp16 stationary/moving operand matrices."""
    p = pred.astype(np.float64)
    g = gt.astype(np.float64)
    ph, pl = _split16(p)  # [N,3] each
    gh, gl = _split16(g)
    pt = ph.astype(np.float64) + pl.astype(np.float64)
    gt_ = gh.astype(np.float64) + gl.astype(np.float64)
    pn = (pt * pt).sum(-1)  # [N]
    gn = (gt_ * gt_).sum(-1)  # [M]
    pnh, pnl = _split16(-pn)
    gnh, gnl = _split16(-gn)

    A = np.zeros((KSLOTS, N), np.float16)
    G = np.zeros((KSLOTS, M), np.float16)
    for c in range(3):
        r = 3 * c
        # (ph+pl)*(gh+gl) ~= ph*gh + ph*gl + pl*gh  (pl*gl ~ 2^-22, dropped)
        A[r + 0] = 2.0 * ph[:, c]
        A[r + 1] = 2.0 * ph[:, c]
        A[r + 2] = 2.0 * pl[:, c]
        G[r + 0] = gh[:, c]
        G[r + 1] = gl[:, c]
        G[r + 2] = gh[:, c]
    A[9] = pnh
    A[10] = pnl
    G[9] = 1.0
    G[10] = 1.0
    A[11] = 1.0
    A[12] = 1.0
    G[11] = gnh
    G[12] = gnl
    return A, G


def _get_nc():
    if "nc" not in _CACHE:
        _CACHE["nc"] = _build_nc()
    return _CACHE["nc"]


def kernel(pred_pc, gt_pc):
    global LAST_RESULT
    pred_pc = np.asarray(pred_pc)
    gt_pc = np.asarray(gt_pc)
    nc = _get_nc()
    in_maps = []
    for b in range(B):
        A, G = _prep(pred_pc[b], gt_pc[b])
        in_maps.append({"A": A, "G": G})
    res = run_bass_kernel_spmd(nc, in_maps, list(range(B)))
    LAST_RESULT = res
    losses = []
    for b in range(B):
        r1 = np.asarray(res.results[b]["R1OUT"], np.float32)  # [P, NTILE]
        c2 = np.asarray(res.results[b]["C2OUT"], np.float32).max(axis=0)  # [4096]
        d1 = np.sqrt(np.maximum(-r1, 0.0)).reshape(-1)  # [4096]
        d2 = np.sqrt(np.maximum(-c2, 0.0))  # [4096]
        loss = 0.0
        for d in (d1, d2):
            topk = np.partition(d, d.size - K1)[d.size - K1 :]
            loss += d.mean() + WEIGHT * topk.mean()
        losses.append(loss)
    return np.array(np.mean(losses), dtype=np.float32)
